# revision 68
# baseline (speedup 1.0000x reference)
"""Trainium2 Bass kernel for nn_CLF_block (channel-attention block).

Reference computation (per batch item i, with x = concat([a,b], ch) in [256, N],
N = H*W = 16384):
    z  = w1 x + b1 1^T
    q  = w2 z + b2 1^T ;  k = w3 z + b3 1^T ;  v = w4 z + b4 1^T
    qk = q k^T ; attn = softmax(qk, -1) ; out = attn v

Host-side weight folding (free: runs in numpy inside kernel()):
    q = A x + p 1^T   with A = w2 w1, p = w2 b1 + b2
    k = B x + r 1^T   with B = w3 w1, r = w3 b1 + b3
    v = D x + t 1^T   with D = w4 w1, t = w4 b1 + b4
so with Gx = x x^T (symmetric) and sx = x 1:
    qk   = A Gx B^T + (A sx) r^T + p (B sx)^T + N p r^T
    attn = softmax(qk)
    out  = (attn D) x + (attn t) 1^T = W x + c0 1^T

Numerics: x is fp16; Gx accumulates fp16 products in f32 (PSUM). The
A.Gx.B^T sandwich runs entirely in fp16 via hi/lo compensated splits
(M = M_hi + M_lo, both fp16; lo x lo cross terms dropped), which is
~3.5x faster on the PE than the f32 LOW_HIGH mode it replaces and
numerically equivalent (measured 3.1e-3 vs the f64 reference, same as
the f32 version; tolerance 2e-2). A and B ship as host-computed hi/lo
fp16 pairs; Gx and S = Gx B^T are split on-chip, with the lo part
computed by the DVE directly against the f32 PSUM. The softmax
normalization (1/denom) is NOT applied to attn; instead the pass-2
psum drains apply out = rden * (Wun x) + rden*c0un via the activation
scale operand / DVE tensor_scalar mult-add, removing the diag(1/denom)
build and its serial chain from the critical path. The N p r^T term
ships precomputed in f32 and is added in-psum by the DVE.

Schedule: piece 0 of the x^T stream rides the scalar HWDGE queue (in
halves, ahead of the EXP-table preload) so the sync queue starts
pieces 1..15 immediately — queue dispatches cost ~610ns each. The sync
queue then carries the two constant tensors (slotted at pieces 10/13)
and the resident x; early output stores ride SWDGE (GpSimd), later
ones the sync queue, and the last tile pair flushes split across both.
A short PE warmup plus fill matmuls around the softmax keep the PE
clock from gating (cold/regated matmuls run at half speed, measured
427ns vs 216ns).

Sharding: data-parallel over batch, one batch item per NeuronCore (B=8).
"""

import sys

if "/opt/trn_rl_repo" not in sys.path:
    sys.path.insert(0, "/opt/trn_rl_repo")

from contextlib import ExitStack

import numpy as np

import concourse.bass as bass
import concourse.mybir as mybir
import concourse.tile as tile
from concourse import bacc
from concourse.bass_utils import run_bass_kernel_spmd

F32 = mybir.dt.float32
F16 = mybir.dt.float16
BF16 = mybir.dt.bfloat16
P = 128            # partitions / channel block
C = 256            # channels
NPIX = 128 * 128   # spatial positions per batch item
NPIECE = 16        # x^T stream pieces
CH_PP = 8          # gram chunks per piece
NCHUNK = NPIECE * CH_PP   # 128 gram chunks
XCHUNK = NPIX // 4        # resident x DMA chunk width
OUTW = 2048        # output staging tile width
NT = 512           # pass-2 psum tile width

# packed fp16 constant layout (columns)
# early constants (needed right at gram end): identities + B^T hi/lo
WA_IDH = 0         # identity, fp16 (for the Gx block transposes)
WA_ID = 128        # identity, bf16 bit pattern (bitcast on chip)
WA_BH = 256        # B^T hi (two row-blocks)
WA_BL = 768        # B^T lo
WA_W = 1280
# late constants (needed from qk onward)
WB_D = 0           # D (two row-blocks side by side)
WB_T = 512         # t (two columns)
WB_P = 514         # p row
WB_R = 770         # r row
WB_AH = 1026       # A^T hi (two row-blocks)
WB_AL = 1538       # A^T lo
WB_PL = 2050       # p lo row
WB_NRH = 2306      # N*r hi row
WB_NRL = 2562      # N*r lo row
WB_W = 2818


def _emit(nc, tc, ctx, d_in, d_out):
    """Emit the Tile program for one core (one batch item)."""
    xht_d, xh_d = d_in["xht"], d_in["xh"]
    wc16a_d, wc16b_d = d_in["wc16a"], d_in["wc16b"]
    out_d = d_out["out"]

    const = ctx.enter_context(tc.tile_pool(name="const", bufs=1))
    xpool = ctx.enter_context(tc.tile_pool(name="xpool", bufs=1))

    # --- PE warm-up: a short burst so the clock ungates; the gram then
    # rides the tail of the ramp.
    warm16 = const.tile([P, C], F16, name="warm16", tag="warm16")
    nc.vector.memset(warm16, 0.0)
    with tc.tile_pool(name="warm_ps0", bufs=1, space="PSUM") as wps0:
        wp = wps0.tile([P, C], F32, name="wp0", tag="wp0")
        for _ in range(6):
            nc.tensor.matmul(wp, warm16[:, 0:P], warm16,
                             start=True, stop=True)

    # constants ride the otherwise-idle SWDGE (gpsimd) queue, delayed to
    # mid-stream so their HBM traffic never starves the early pieces
    # that pace the gram start
    wc16a = const.tile([P, WA_W], F16, name="wc16a", tag="wc16a")
    wc16b = const.tile([P, WB_W], F16, name="wc16b", tag="wc16b")

    btH = [wc16a[:, WA_BH + k * C:WA_BH + (k + 1) * C] for k in range(2)]
    btL = [wc16a[:, WA_BL + k * C:WA_BL + (k + 1) * C] for k in range(2)]
    ident_sb = wc16a[:, WA_ID:WA_ID + P].bitcast(BF16)
    ident16 = wc16a[:, WA_IDH:WA_IDH + P]
    atH = [wc16b[:, WB_AH + k * C:WB_AH + (k + 1) * C] for k in range(2)]
    atL = [wc16b[:, WB_AL + k * C:WB_AL + (k + 1) * C] for k in range(2)]
    pl_row = wc16b[0:1, WB_PL:WB_PL + C]
    nrh_row = wc16b[0:1, WB_NRH:WB_NRH + C]
    nrl_row = wc16b[0:1, WB_NRL:WB_NRL + C]
    dm_ = [wc16b[:, WB_D + k * C:WB_D + (k + 1) * C] for k in range(2)]
    tcol = [wc16b[:, WB_T + k:WB_T + k + 1] for k in range(2)]
    p_row = wc16b[0:1, WB_P:WB_P + C]
    r_row = wc16b[0:1, WB_R:WB_R + C]

    # --- pass-1 stream. Queue dispatches cost ~610ns each, so piece 0
    # rides the otherwise-idle SCALAR HWDGE queue (in halves, emitted
    # before the EXP-table preload) while the sync queue starts on
    # pieces 1..15 immediately.
    xtp = ctx.enter_context(tc.tile_pool(name="xt_sb", bufs=8))
    H_PP = CH_PP // 2
    xh0 = [
        const.tile([P, H_PP, C + 1], F16, name=f"xh0_{h}", tag=f"xh0_{h}")
        for h in range(2)
    ]

    warm_act = const.tile([P, 4], F32, name="warm_act", tag="warm_act")

    xht_p = [None]
    for i in range(1, 4):
        xt = xtp.tile([P, CH_PP, C + 1], F16, name="xht_p", tag="xht_p")
        nc.sync.dma_start(out=xt, in_=xht_d[i])
        xht_p.append(xt)

    # --- pass 1: Gx = xh xh^T (fp16 products, f32 accumulation) ----------
    # TRIANGLE gram: Gx is symmetric, so block row 0 accumulates the
    # full [Gx(0,0) Gx(0,1) | sx0] (F=257) while block row 1 only
    # accumulates [Gx(1,1) | sx1] (F=129) — 25% less PE time. The
    # missing Gx(1,0) block is the transpose of the quantized (0,1)
    # block, rebuilt by two tiny PE transposes after the hi/lo split.
    gx_hi = [
        const.tile([P, C // (1 + b)], F16, name=f"gx_hi{b}",
                   tag=f"gx_hi{b}")
        for b in range(2)
    ]
    gx_lo = [
        const.tile([P, C // (1 + b)], F16, name=f"gx_lo{b}",
                   tag=f"gx_lo{b}")
        for b in range(2)
    ]
    with tc.tile_pool(name="gx_ps", bufs=1, space="PSUM") as gxp:
        shh = [
            gxp.tile([P, C + 1 - b * P], F32, name=f"shh{b}", tag=f"shh{b}")
            for b in range(2)
        ]
        # piece 0 (on the scalar queue) is summed LAST: its transfer
        # contends with the sync stream at t~8us, and the gram sum is
        # order-free, so the PE starts on piece 1 (which lands first)
        # and piece 0's halves get ~30us of slack instead of stalling
        # the very first matmuls.
        for seq, i in enumerate(list(range(1, NPIECE)) + [0]):
            if 4 <= i:
                xt = xtp.tile([P, CH_PP, C + 1], F16, name="xht_p",
                              tag="xht_p")
                nc.sync.dma_start(out=xt, in_=xht_d[i])
                xht_p.append(xt)
            if seq == 1:
                # preload the EXP activation table (1.3us ACT_TABLE_LOAD)
                # on the scalar queue well before the softmax needs it
                nc.scalar.activation(
                    out=warm_act, in_=warm16[:, 0:4],
                    func=mybir.ActivationFunctionType.Exp, bias=0.0)
            elif seq == NPIECE - 1:
                # everything that is NOT gram-critical loads on the SAME
                # sync queue, in exact consumption order, AFTER the
                # pieces: the FIFO self-throttles, so none of it can
                # steal HBM from the pieces that pace the gram. (Queue
                # position, not emission slot, controls dispatch time —
                # a separate queue would fire these immediately.)
                for h in range(2):
                    nc.sync.dma_start(
                        out=xh0[h],
                        in_=xht_d[0][:, h * H_PP:(h + 1) * H_PP, :])
                nc.sync.dma_start(out=wc16a, in_=wc16a_d[:, :])
                nc.sync.dma_start(out=wc16b, in_=wc16b_d[:, :])
            for g in range(CH_PP):
                ch = seq * CH_PP + g
                src_t = (xh0[g // H_PP][:, g % H_PP] if i == 0
                         else xht_p[i][:, g])
                for b in range(2):
                    nc.tensor.matmul(shh[b],
                                     src_t[:, b * P:(b + 1) * P],
                                     src_t[:, b * P:C + 1],
                                     start=(ch == 0),
                                     stop=(ch == NCHUNK - 1))
        # resident x for pass 2, after the stream on the same queue, in
        # quarter chunks ordered exactly as pass 2 consumes them so the
        # supply stays just ahead of the matmuls even while stores
        # contend for HBM
        xs = [[], []]
        for j in range(4):
            for k in range(2):
                xr = xpool.tile([P, XCHUNK], F16, name=f"x{k}_{j}",
                                tag=f"x{k}_{j}")
                nc.sync.dma_start(
                    out=xr,
                    in_=xh_d[k * P:(k + 1) * P,
                             j * XCHUNK:(j + 1) * XCHUNK])
                xs[k].append(xr)
        # Gx hi/lo compensated split, his first (they gate the S start)
        # split across both engines; los and the small sx columns follow
        # on the DVE while the PE runs the S hi-terms.
        nc.scalar.activation(out=gx_hi[0], in_=shh[0][:, 0:C],
                             func=mybir.ActivationFunctionType.Identity,
                             bias=0.0, scale=1.0)
        nc.vector.tensor_copy(gx_hi[1], shh[1][:, 0:P])
        nc.vector.tensor_sub(gx_lo[0], shh[0][:, 0:C], gx_hi[0])
        nc.vector.tensor_sub(gx_lo[1], shh[1][:, 0:P], gx_hi[1])
        sxc = []
        for b in range(2):
            sc = const.tile([P, 1], F16, name=f"sxc{b}", tag=f"sxc{b}")
            nc.vector.tensor_copy(sc, shh[b][:, C - b * P:C - b * P + 1])
            sxc.append(sc)
        # Rebuild Gx(1,0) = Gx(0,1)^T (hi and lo) with two tiny PE
        # transposes (psum banks still free in this pool's scope)
        gxt_hi = const.tile([P, P], F16, name="gxt_hi", tag="gxt_hi")
        gxt_lo = const.tile([P, P], F16, name="gxt_lo", tag="gxt_lo")
        for src, dst, tg in ((gx_hi[0][:, P:C], gxt_hi, "tpsTh"),
                             (gx_lo[0][:, P:C], gxt_lo, "tpsTl")):
            tpsT = gxp.tile([P, P], F16, name=tg, tag=tg)
            nc.tensor.transpose(tpsT, src, ident16)
            nc.vector.tensor_copy(dst, tpsT)

    # --- 256x256 algebra (all fp16 on the PE) ----------------------------
    alg = const
    with tc.tile_pool(name="alg_ps", bufs=3, space="PSUM") as ap:
        wp_alg = ap.tile([P, C], F32, name="wp_alg", tag="warm", bufs=1)

        # S = Gx B^T via hi/lo fp16 (lo x lo dropped); b=1's group first —
        # it doesn't need the transposed Gx(1,0) block.
        s_hi = [None, None]
        s_lo = [None, None]

        def s_block(b, lhsT_hi, lhsT_lo):
            sps = ap.tile([P, C], F32, name="sps", tag="alg")
            j = 0
            for lh, rh in ((lhsT_hi, btH), (lhsT_hi, btL),
                           (lhsT_lo, btH)):
                for k in range(2):
                    nc.tensor.matmul(sps, lh[k], rh[k],
                                     start=(j == 0), stop=(j == 5))
                    j += 1
            sh = alg.tile([P, C], F16, name=f"s_hi{b}", tag=f"s_hi{b}")
            sl = alg.tile([P, C], F16, name=f"s_lo{b}", tag=f"s_lo{b}")
            # his on ACT (it idles between them), los on DVE — the two
            # split chains then run fully in parallel
            nc.scalar.activation(
                out=sh, in_=sps,
                func=mybir.ActivationFunctionType.Identity,
                bias=0.0, scale=1.0)
            nc.vector.tensor_sub(sl, sps, sh)
            s_hi[b] = sh
            s_lo[b] = sl

        s_block(1, [gx_hi[0][:, P:C], gx_hi[1]],
                [gx_lo[0][:, P:C], gx_lo[1]])
        s_block(0, [gx_hi[0][:, 0:P], gxt_hi],
                [gx_lo[0][:, 0:P], gxt_lo])

        # asx_row = (A sx)^T, bsx_row = (B sx)^T  (fp16 matvecs, hi+lo);
        # emitted AFTER S so they fill the S-drain window on the PE
        # instead of delaying S
        asx_row = alg.tile([1, C], F16, name="asx_row", tag="asx_row")
        bsx_row = alg.tile([1, C], F16, name="bsx_row", tag="bsx_row")
        for dst, wh, wl in ((asx_row, atH, atL), (bsx_row, btH, btL)):
            vps = ap.tile([1, C], F32, name="vps", tag="algsmall", bufs=2)
            for j, wt in enumerate((wh[0], wh[1], wl[0], wl[1])):
                nc.tensor.matmul(vps, sxc[j % 2], wt,
                                 start=(j == 0), stop=(j == 3))
            nc.vector.tensor_copy(dst, vps)

        # qk = A S + asx r^T + p bsx^T (+ N p r^T via f32 qkc); softmax
        rden = []
        attn_sb = []
        for b in range(2):
            qkps = ap.tile([P, C], F32, name="qkps", tag="alg")
            j = 0
            for lh, rh in ((atH, s_hi), (atL, s_hi), (atH, s_lo)):
                for k in range(2):
                    nc.tensor.matmul(qkps, lh[k][:, b * P:(b + 1) * P],
                                     rh[k], start=(j == 0), stop=False)
                    j += 1
            nc.tensor.matmul(qkps, asx_row[:, b * P:(b + 1) * P], r_row,
                             start=False, stop=False)
            nc.tensor.matmul(qkps, p_row[:, b * P:(b + 1) * P], bsx_row,
                             start=False, stop=False)
            # the N p r^T term (~+-57, needs > fp16 single precision) as
            # three fp16 hi/lo rank-1 matmuls (~165ns each) — cheaper
            # than the old in-psum DVE add and off the DVE serial chain
            nc.tensor.matmul(qkps, p_row[:, b * P:(b + 1) * P], nrh_row,
                             start=False, stop=False)
            nc.tensor.matmul(qkps, p_row[:, b * P:(b + 1) * P], nrl_row,
                             start=False, stop=False)
            nc.tensor.matmul(qkps, pl_row[:, b * P:(b + 1) * P], nrh_row,
                             start=False, stop=True)
            if b == 1:
                # cheap fp16 fills so the PE never idles a full HAM window
                # while the softmax chain runs
                for _ in range(6):
                    nc.tensor.matmul(wp_alg, warm16[:, 0:P], warm16,
                                     start=True, stop=True)

            negmax = alg.tile([P, 1], F32, name=f"negmax{b}", tag=f"nm{b}")
            nc.vector.tensor_reduce(
                out=negmax, in_=qkps, op=mybir.AluOpType.max,
                axis=mybir.AxisListType.X, negate=True,
            )
            expq = alg.tile([P, C], BF16, name=f"expq{b}", tag=f"expq{b}")
            denom = alg.tile([P, 1], F32, name=f"denom{b}", tag=f"dn{b}")
            nc.scalar.activation(
                out=expq, in_=qkps, func=mybir.ActivationFunctionType.Exp,
                bias=negmax, scale=1.0, accum_out=denom,
            )
            rd = alg.tile([P, 1], F32, name=f"rden{b}", tag=f"rd{b}")
            nc.vector.reciprocal(rd, denom)
            rden.append(rd)
            attn_sb.append(expq)

        # attn^T (unnormalized) via 4 transpose matmuls, stored fp16; the
        # softmax normalization is applied later in the pass-2 drains.
        attnT_sb = [
            alg.tile([P, C], F16, name=f"attnT{j}", tag=f"attnT{j}")
            for j in range(2)
        ]
        for b in range(2):
            expq_b = attn_sb[b]
            for j in range(2):
                tps = ap.tile([P, P], F32, name="tps", tag="algtp", bufs=2)
                nc.tensor.matmul(tps, expq_b[:, j * P:(j + 1) * P],
                                 ident_sb, start=True, stop=True)
                if j == 0:
                    nc.scalar.activation(
                        out=attnT_sb[j][:, b * P:(b + 1) * P], in_=tps,
                        func=mybir.ActivationFunctionType.Identity,
                        bias=0.0, scale=1.0)
                else:
                    nc.vector.tensor_copy(
                        attnT_sb[j][:, b * P:(b + 1) * P], tps)

        for _ in range(4):
            nc.tensor.matmul(wp_alg, warm16[:, 0:P], warm16,
                             start=True, stop=True)

        # Wun^T = D^T attn^T (fp16, unnormalized), cast per block so pass 2
        # can start before the c0 matvecs retire
        wt16 = []
        for b in range(2):
            wps = ap.tile([P, C], F32, name="wps", tag="alg")
            for k in range(2):
                nc.tensor.matmul(wps, dm_[k][:, b * P:(b + 1) * P],
                                 attnT_sb[k], start=(k == 0), stop=(k == 1))
            wt_ = alg.tile([P, C], F16, name=f"wt16_{b}", tag=f"wt16_{b}")
            if b == 0:
                nc.scalar.activation(
                    out=wt_, in_=wps,
                    func=mybir.ActivationFunctionType.Identity,
                    bias=0.0, scale=1.0)
            else:
                nc.vector.tensor_copy(wt_, wps)
            wt16.append(wt_)

        # c0 = rden * (attn_un t) per q block (normalization folded into
        # the drain's multiply)
        c0_col = []
        for b in range(2):
            cps = ap.tile([P, 1], F32, name="cps", tag="algsmall", bufs=2)
            for k in range(2):
                nc.tensor.matmul(cps, attnT_sb[k][:, b * P:(b + 1) * P],
                                 tcol[k], start=(k == 0), stop=(k == 1))
            ct = alg.tile([P, 1], F32, name=f"c0_col{b}", tag=f"c0_col{b}")
            nc.vector.tensor_scalar(
                out=ct, in0=cps, scalar1=rden[b], scalar2=None,
                op0=mybir.AluOpType.mult,
            )
            c0_col.append(ct)

    # --- pass 2: out = rden*(Wun x) + c0, fp16 ---------------------------
    with tc.tile_pool(name="o_ps", bufs=8, space="PSUM") as ops, \
         tc.tile_pool(name="o_sb", bufs=6) as osb:
        nsub = OUTW // NT
        ngrp = NPIX // OUTW
        for i in range(ngrp):
            xj = (i * OUTW) // XCHUNK
            xo = (i * OUTW) % XCHUNK
            for b in range(2):
                ot = osb.tile([P, OUTW], F16, name="ot", tag="ot")
                for t in range(nsub):
                    pst = ops.tile([P, NT], F32, name="pst", tag="pst")
                    for k in range(2):
                        nc.tensor.matmul(
                            pst,
                            wt16[k][:, b * P:(b + 1) * P],
                            xs[k][xj][:, xo + t * NT:xo + (t + 1) * NT],
                            start=(k == 0),
                            stop=(k == 1),
                        )
                    # psum drain: out = rden*psum + c0, fp16 cast; split
                    # across the otherwise-idle Scalar and Vector engines
                    if t % 2 == 0:
                        nc.scalar.activation(
                            out=ot[:, t * NT:(t + 1) * NT], in_=pst,
                            func=mybir.ActivationFunctionType.Identity,
                            bias=c0_col[b], scale=rden[b],
                        )
                    else:
                        nc.vector.tensor_scalar(
                            out=ot[:, t * NT:(t + 1) * NT], in0=pst,
                            scalar1=rden[b], scalar2=c0_col[b],
                            op0=mybir.AluOpType.mult,
                            op1=mybir.AluOpType.add,
                        )
                # early tiles ride SWDGE (sync still drains the input);
                # the second half splits by block across both queues; the
                # final pair flushes in halves on sync (HWDGE) so the slow
                # SWDGE path never gates the tail. Never the scalar queue
                # (head-of-line risk for the psum drains).
                if i == ngrp - 1:
                    # both final tiles on sync: SWDGE's descriptor flush
                    # (a ~3.5us GpSimd DRAIN) would otherwise gate the
                    # epilogue
                    for hh in range(2):
                        eng = nc.sync
                        eng.dma_start(
                            out=out_d[b * P:(b + 1) * P,
                                      i * OUTW + hh * (OUTW // 2):
                                      i * OUTW + (hh + 1) * (OUTW // 2)],
                            in_=ot[:, hh * (OUTW // 2):(hh + 1) * (OUTW // 2)],
                        )
                else:
                    # first two groups all-SWDGE (sync is still loading
                    # the resident x), then b0 -> SWDGE, b1 -> sync so
                    # both queues share the store load evenly
                    if i < 2:
                        seng = nc.gpsimd
                    else:
                        seng = nc.gpsimd if b == 0 else nc.sync
                    seng.dma_start(
                        out=out_d[b * P:(b + 1) * P,
                                  i * OUTW:(i + 1) * OUTW],
                        in_=ot,
                    )


def build_program(enable_asserts=False):
    nc = bacc.Bacc(
        "TRN2",
        target_bir_lowering=False,
        debug=False,
        enable_asserts=enable_asserts,
        num_devices=8,
    )
    d_in = {
        "xht": nc.dram_tensor("xht", [NPIECE, P, CH_PP, C + 1],
                              F16, kind="ExternalInput").ap(),
        "xh": nc.dram_tensor("xh", [C, NPIX], F16,
                             kind="ExternalInput").ap(),
        "wc16a": nc.dram_tensor("wc16a", [P, WA_W], F16,
                                kind="ExternalInput").ap(),
        "wc16b": nc.dram_tensor("wc16b", [P, WB_W], F16,
                                kind="ExternalInput").ap(),
    }
    d_out = {
        "out": nc.dram_tensor("out", [C, NPIX], F16,
                              kind="ExternalOutput").ap(),
    }
    with tile.TileContext(nc) as tc, ExitStack() as ctx:
        _emit(nc, tc, ctx, d_in, d_out)
    nc.compile()
    return nc


def make_in_maps(a, b, w1, b1, w2, b2, w3, b3, w4, b4):
    N = NPIX
    f = np.float32
    f16 = np.float16
    f64 = np.float64
    A = (w2.astype(f64) @ w1.astype(f64))
    B_ = (w3.astype(f64) @ w1.astype(f64))
    D = (w4.astype(f64) @ w1.astype(f64))
    p = (w2.astype(f64) @ b1.astype(f64) + b2)
    r = (w3.astype(f64) @ b1.astype(f64) + b3)
    t = (w4.astype(f64) @ b1.astype(f64) + b4)

    def blocks2(m):  # [256, X] -> [128, 2X] (two row-blocks side by side)
        return np.concatenate([m[0:P, :], m[P:2 * P, :]], axis=1)

    def hilo(m):
        hi = m.astype(f16)
        lo = (m - hi.astype(f64)).astype(f16)
        return hi, lo

    atHi, atLo = hilo(A.T)
    btHi, btLo = hilo(B_.T)

    import ml_dtypes
    wc16a = np.zeros((P, WA_W), f16)
    wc16a[:, WA_IDH:WA_IDH + P] = np.eye(P, dtype=f16)
    ident_bf = np.eye(P, dtype=ml_dtypes.bfloat16)
    wc16a[:, WA_ID:WA_ID + P] = ident_bf.view(np.uint16).view(f16)
    wc16a[:, WA_BH:WA_BH + 2 * C] = blocks2(btHi)
    wc16a[:, WA_BL:WA_BL + 2 * C] = blocks2(btLo)
    wc16b = np.zeros((P, WB_W), f16)
    wc16b[:, WB_D:WB_D + 2 * C] = blocks2(D.astype(f16))
    wc16b[:, WB_T:WB_T + 2] = t.astype(f16).reshape(2, P).T
    wc16b[0, WB_P:WB_P + C] = p.astype(f16)
    wc16b[0, WB_R:WB_R + C] = r.astype(f16)
    wc16b[:, WB_AH:WB_AH + 2 * C] = blocks2(atHi)
    wc16b[:, WB_AL:WB_AL + 2 * C] = blocks2(atLo)
    p_lo = (p - p.astype(f16).astype(f64))
    nr = N * r
    nr_lo = (nr - nr.astype(f16).astype(f64))
    wc16b[0, WB_PL:WB_PL + C] = p_lo.astype(f16)
    wc16b[0, WB_NRH:WB_NRH + C] = nr.astype(f16)
    wc16b[0, WB_NRL:WB_NRL + C] = nr_lo.astype(f16)

    B = a.shape[0]
    in_maps = []
    for i in range(B):
        x = np.concatenate([a[i].reshape(P, N), b[i].reshape(P, N)], axis=0)
        xh = x.astype(np.float16)
        xht = np.ascontiguousarray(
            xh.T.reshape(NPIECE, CH_PP, P, C).transpose(0, 2, 1, 3))
        ones = np.ones((NPIECE, P, CH_PP, 1), np.float16)
        xht = np.ascontiguousarray(np.concatenate([xht, ones], axis=3))
        in_maps.append({
            "xht": xht,
            "xh": xh,
            "wc16a": wc16a,
            "wc16b": wc16b,
        })
    return in_maps


_CACHE = {}


def kernel(a, b, w1, b1, w2, b2, w3, b3, w4, b4, _trace=False):
    a = np.asarray(a, dtype=np.float32)
    b = np.asarray(b, dtype=np.float32)
    args = [np.asarray(t, dtype=np.float32)
            for t in (w1, b1, w2, b2, w3, b3, w4, b4)]
    if "nc" not in _CACHE:
        _CACHE["nc"] = build_program()
    nc = _CACHE["nc"]
    in_maps = make_in_maps(a, b, *args)
    res = run_bass_kernel_spmd(nc, in_maps, core_ids=list(range(8)),
                               trace=_trace)
    B, Ch, H, W = a.shape
    out = np.stack([
        r["out"].astype(np.float32).reshape(C, H, W) for r in res.results
    ])
    if _trace:
        _CACHE["last_results"] = res
    return out
